# revision 12
# baseline (speedup 1.0000x reference)
"""ProxyNCA loss on 8 Trainium2 NeuronCores — ACT+DVE split exp, paired
DVE reduces.

loss_i = -G[i,t_i] + log sum_{k != t_i} exp(G[i,k]), G = 2 X Pn^T.
Host prep normalizes/scales/casts proxies; host combine subtracts
exp(pos) and averages in float64.

Per core, the [1024, 12500] G slab is computed by row-packed bf16
matmuls. The row-sum of exp(G) is split across two engines working from
PSUM in parallel: ScalarE handles 1536-wide tiles (plus per-block tails)
with native Exp + fused accumulation; VectorE handles 512-wide tiles
with a Schraudolph bit-trick exp (int32(A*g + B) written into a 1024-wide
scratch, re-read as fp32 through an aliased tensor from
alloc_sbuf_tensor_at). Two consecutive DVE tiles of a block fill the two
scratch halves and share ONE 1024-wide row-sum reduce, halving the DVE
reduce instruction count. The reduce declares the int32 view as an extra
input so the scheduler orders the aliased accesses. Host combines in
float64.
"""

import numpy as np
import ml_dtypes

import concourse.bacc as bacc
import concourse.mybir as mybir
import concourse.tile as tile
from concourse.bass_utils import run_bass_kernel_spmd

F32 = mybir.dt.float32
I32 = mybir.dt.int32
BF16 = mybir.dt.bfloat16
AX = mybir.AxisListType.X
MULT = mybir.AluOpType.mult
ADD = mybir.AluOpType.add
EXP = mybir.ActivationFunctionType.Exp

B, C, D = 1024, 100000, 64
NCORES = 8
CS = C // NCORES          # 12500 classes per core
BS = B // NCORES          # 128 batch rows per core
NBLK = B // 128           # 8 batch blocks of 128 rows
WA = 1536                 # ACT tile width (3 PSUM banks)
WD = 512                  # DVE tile width (1 PSUM bank)

# Schraudolph constants (i = int32(g*SCH_A + SCH_B); bits(i) ~ exp(g),
# zero-mean ratio error over the uniform mantissa fraction).
SCH_C = 0.0575325
SCH_A = float(np.float32(2 ** 23 / np.log(2.0)))
SCH_B = float(np.float32((127.0 - SCH_C) * 2 ** 23))


def _block_segs(m):
    """Per-block column segments: (offset, width, engine). Even DVE seg
    counts per block so every DVE pair shares one 1024-wide reduce.
    type1: 6x1536 A + 6x512 D + 212 A-tail; type2: 5x1536 A + 8x512 D +
    724 A-tail."""
    if m % 2 == 0:
        pat = ["A", "D"] * 6
        tail = 212
    else:
        pat = ["A", "D", "D"] * 3 + ["A", "D", "A", "D"]
        tail = 724
    segs = []
    off = 0
    for e in pat:
        w = WA if e == "A" else WD
        segs.append((off, w, e))
        off += w
    assert off == CS - tail, (m, off)
    segs.append((off, tail, "A"))
    return segs


def _schedule():
    """Emission-order ops: ("A", m, off, w, col) or
    ("D", m, off, hoff, red_col_or_None)."""
    sched = []
    na = nr = 0
    for m in range(NBLK):
        dcnt = 0
        for off, w, e in _block_segs(m):
            if e == "A":
                sched.append(("A", m, off, w, na))
                na += 1
            else:
                hoff = (dcnt % 2) * WD
                red = None
                if dcnt % 2 == 1:
                    red = nr
                    nr += 1
                sched.append(("D", m, off, hoff, red))
                dcnt += 1
        assert dcnt % 2 == 0, (m, dcnt)
    return sched, na, nr


SCHED, NACT, NRED = _schedule()
# block index of each A column / each reduce column (for host combine)
ABLK = [op[1] for op in SCHED if op[0] == "A"]
RBLK = [op[1] for op in SCHED if op[0] == "D" and op[4] is not None]

_CACHE = {}


def _build(nloop=1, norm_f32r=True):
    nc = bacc.Bacc("TRN2", target_bir_lowering=False, debug=False)

    xt_d = nc.dram_tensor("xt", [D, B], BF16, kind="ExternalInput").ap()
    pt_d = nc.dram_tensor("pt", [D, CS], BF16, kind="ExternalInput").ap()
    xsb_d = nc.dram_tensor("xsb", [BS, D], F32, kind="ExternalInput").ap()
    pp_d = nc.dram_tensor("pp", [BS, D], F32, kind="ExternalInput").ap()
    s_d = nc.dram_tensor("s_out", [128, NACT], F32, kind="ExternalOutput").ap()
    d_d = nc.dram_tensor("d_out", [128, NRED], F32, kind="ExternalOutput").ap()
    pos_d = nc.dram_tensor("pos_out", [BS], F32, kind="ExternalOutput").ap()

    # Schraudolph scratch: 1024-wide, int32 write view + fp32 read view of
    # the same SBUF bytes (alloc_sbuf_tensor_at aliasing).
    arena = nc.alloc_sbuf_tensor("schr_arena", [128, 2 * WD], F32)
    addr = nc.lookup_mloc(arena).addr
    sints = nc.alloc_sbuf_tensor_at("schr_i", [128, 2 * WD], I32,
                                    offset=addr).ap()
    fview = nc.alloc_sbuf_tensor_at("schr_f", [128, 2 * WD], F32,
                                    offset=addr).ap()

    with tile.TileContext(nc) as tc:
        with (
            tc.tile_pool(name="res", bufs=1) as res,
            tc.tile_pool(name="sml", bufs=2) as sml,
            tc.tile_pool(name="exps", bufs=1) as exps,
            tc.tile_pool(name="pa", bufs=2, space="PSUM") as pa,
            tc.tile_pool(name="pd", bufs=2, space="PSUM") as pd,
        ):
            xsb = res.tile([BS, D], F32, tag="xsb")
            pp = res.tile([BS, D], F32, tag="pp")
            nc.sync.dma_start(xsb[:], xsb_d[:])
            nc.sync.dma_start(pp[:], pp_d[:])
            xtr2 = res.tile([2 * D, B], BF16, tag="xtr2")
            nc.sync.dma_start(xtr2[0:D, :], xt_d[:])
            nc.sync.dma_start(xtr2[D:2 * D, :], xt_d[:])
            ptn2 = res.tile([2 * D, CS], BF16, tag="ptn2")
            nc.sync.dma_start(ptn2[0:D, :], pt_d[:])
            nc.sync.dma_start(ptn2[D:2 * D, :], pt_d[:])

            def mms(ps, m, off, w):
                """Row-packed 512-col matmul pairs covering [off, off+w)."""
                c0 = 0
                h = 0
                while c0 < w:
                    cw = min(512, w - c0)
                    lo = D * h
                    nc.tensor.matmul(ps[:, c0:c0 + cw],
                                     xtr2[lo:lo + D, 128 * m:128 * (m + 1)],
                                     ptn2[lo:lo + D, off + c0:off + c0 + cw],
                                     start=True, stop=True,
                                     tile_position=(lo, 0))
                    h ^= 1
                    c0 += cw

            def body():
                # positive term pos = 2 x.p_hat_t
                xp = sml.tile([BS, D], F32, tag="xp")
                nc.vector.tensor_tensor(xp[:], xsb[:], pp[:], op=MULT)
                pos = sml.tile([BS, 1], F32, tag="pos")
                nc.vector.reduce_sum(pos[:], xp[:], axis=AX)
                nc.sync.dma_start(pos_d[:], pos[:, 0])

                sums = sml.tile([128, NACT], F32, tag="sums")
                dsums = sml.tile([128, NRED], F32, tag="dsums")
                for op in SCHED:
                    if op[0] == "A":
                        _, m, off, w, j = op
                        ps = pa.tile([128, WA], F32, tag="ps")
                        mms(ps, m, off, w)
                        ex = exps.tile([128, WA], BF16, tag="ex")
                        nc.scalar.activation(ex[:, 0:w], ps[:, 0:w], EXP,
                                             accum_out=sums[:, j:j + 1])
                    else:
                        _, m, off, hoff, red = op
                        psd = pd.tile([128, WD], F32, tag="psd")
                        mms(psd, m, off, WD)
                        nc.vector.tensor_scalar(sints[:, hoff:hoff + WD],
                                                psd[:], SCH_A, SCH_B,
                                                op0=MULT, op1=ADD)
                        if red is not None:
                            r = nc.vector.reduce_sum(
                                dsums[:, red:red + 1],
                                fview[:, 0:2 * WD], axis=AX)
                            # fview aliases sints' bytes but is a different
                            # tensor; declare the int view as an extra input
                            # so RAW (ts->reduce) and WAR (reduce->next ts)
                            # ordering holds.
                            ri = r.ins
                            ri.ins = list(ri.ins) + [
                                nc.vector.lower_ap(sints[:, 0:2 * WD])]
                nc.sync.dma_start(s_d[:], sums[:])
                nc.sync.dma_start(d_d[:], dsums[:])

            if nloop == 1:
                body()
            else:
                with tc.For_i(0, nloop, 1):
                    body()

    nc.compile()
    return nc


def _get_nc(nloop=1, norm_f32r=True):
    key = (nloop, norm_f32r)
    if key not in _CACHE:
        _CACHE[key] = _build(nloop, norm_f32r)
    return _CACHE[key]


def _in_maps(xs, ts, proxies):
    xs = np.ascontiguousarray(np.asarray(xs), dtype=np.float32)
    proxies = np.ascontiguousarray(np.asarray(proxies), dtype=np.float32)
    ts = np.asarray(ts).astype(np.int64)
    norms = np.sqrt((proxies.astype(np.float64) ** 2).sum(1))
    p2 = proxies * (2.0 / np.maximum(norms, 1e-12))[:, None].astype(np.float32)
    pt_all = np.ascontiguousarray(p2.T.astype(ml_dtypes.bfloat16))
    xt = np.ascontiguousarray(xs.T.astype(ml_dtypes.bfloat16))
    ppos = p2[ts]
    maps = []
    for c in range(NCORES):
        maps.append({
            "xt": xt,
            "pt": np.ascontiguousarray(pt_all[:, c * CS:(c + 1) * CS]),
            "xsb": xs[c * BS:(c + 1) * BS],
            "pp": np.ascontiguousarray(ppos[c * BS:(c + 1) * BS]),
        })
    return maps


def _combine(results, ts=None):
    s = np.zeros(B, dtype=np.float64)
    pos = np.zeros(B, dtype=np.float64)
    for c in range(NCORES):
        so = results[c]["s_out"].astype(np.float64)   # [128, NACT]
        do = results[c]["d_out"].astype(np.float64)   # [128, NRED]
        acc = np.zeros((NBLK, 128), dtype=np.float64)
        for j, m in enumerate(ABLK):
            acc[m] += so[:, j]
        for j, m in enumerate(RBLK):
            acc[m] += do[:, j]
        s += acc.reshape(B)
        pos[c * BS:(c + 1) * BS] = results[c]["pos_out"].astype(np.float64)
    r = s - np.exp(pos)
    loss = np.mean(-pos + np.log(r))
    return np.asarray(loss, dtype=np.float32)


def kernel(xs, ts, proxies):
    nc = _get_nc()
    maps = _in_maps(xs, ts, proxies)
    results = run_bass_kernel_spmd(nc, maps, list(range(NCORES))).results
    return _combine(results, ts)


if __name__ == "__main__":
    rng = np.random.default_rng(0)
    xs = rng.standard_normal((B, D)).astype(np.float32)
    ts = rng.integers(0, C, B)
    proxies = rng.standard_normal((C, D)).astype(np.float32)
    print(kernel(xs=xs, ts=ts, proxies=proxies))


# revision 14
# speedup vs baseline: 1.3317x; 1.3317x over previous
"""ProxyNCA loss on 8 Trainium2 NeuronCores — ACT+DVE split exp, paired
DVE reduces.

loss_i = -G[i,t_i] + log sum_{k != t_i} exp(G[i,k]), G = 2 X Pn^T.
Host prep normalizes/scales/casts proxies; host combine subtracts
exp(pos) and averages in float64.

Per core, the [1024, 12500] G slab is computed by row-packed bf16
matmuls. The row-sum of exp(G) is split across two engines working from
PSUM in parallel: ScalarE handles 1536-wide tiles (plus per-block tails)
with native Exp + fused accumulation; VectorE handles 512-wide tiles
with a Schraudolph bit-trick exp (int32(A*g + B) written into a 1024-wide
scratch, re-read as fp32 through an aliased tensor from
alloc_sbuf_tensor_at). Two consecutive DVE tiles of a block fill the two
scratch halves and share ONE 1024-wide row-sum reduce, halving the DVE
reduce instruction count. The reduce declares the int32 view as an extra
input so the scheduler orders the aliased accesses. Host combines in
float64.
"""

import numpy as np
import ml_dtypes

import concourse.bacc as bacc
import concourse.mybir as mybir
import concourse.tile as tile
from concourse.bass_utils import run_bass_kernel_spmd

F32 = mybir.dt.float32
I32 = mybir.dt.int32
BF16 = mybir.dt.bfloat16
AX = mybir.AxisListType.X
MULT = mybir.AluOpType.mult
ADD = mybir.AluOpType.add
EXP = mybir.ActivationFunctionType.Exp

B, C, D = 1024, 100000, 64
NCORES = 8
CS = C // NCORES          # 12500 classes per core
BS = B // NCORES          # 128 batch rows per core
NBLK = B // 128           # 8 batch blocks of 128 rows
WA = 1536                 # ACT tile width (3 PSUM banks)
WD = 512                  # DVE tile width (1 PSUM bank)

# Schraudolph constants (i = int32(g*SCH_A + SCH_B); bits(i) ~ exp(g),
# zero-mean ratio error over the uniform mantissa fraction).
SCH_C = 0.0575325
SCH_A = float(np.float32(2 ** 23 / np.log(2.0)))
SCH_B = float(np.float32((127.0 - SCH_C) * 2 ** 23))


def _block_segs(m):
    """Per-block column segments: (offset, width, engine). Even DVE seg
    counts per block so every DVE pair shares one 1024-wide reduce.
    type1: 6x1536 A + 6x512 D + 212 A-tail; type2: 5x1536 A + 8x512 D +
    724 A-tail."""
    if m % 2 == 0:
        pat = ["A", "D"] * 6
        tail = 212
    else:
        pat = ["A", "D", "D"] * 3 + ["A", "D", "A", "D"]
        tail = 724
    segs = []
    off = 0
    for e in pat:
        w = WA if e == "A" else WD
        segs.append((off, w, e))
        off += w
    assert off == CS - tail, (m, off)
    segs.append((off, tail, "A"))
    return segs


def _schedule():
    """Emission-order ops: ("A", m, off, w, col) or
    ("D", m, off, hoff, red_col_or_None)."""
    sched = []
    na = nr = 0
    for m in range(NBLK):
        dcnt = 0
        for off, w, e in _block_segs(m):
            if e == "A":
                sched.append(("A", m, off, w, na))
                na += 1
            else:
                hoff = (dcnt % 2) * WD
                red = None
                if dcnt % 2 == 1:
                    red = nr
                    nr += 1
                sched.append(("D", m, off, hoff, red))
                dcnt += 1
        assert dcnt % 2 == 0, (m, dcnt)
    return sched, na, nr


SCHED, NACT, NRED = _schedule()
# block index of each A column / each reduce column (for host combine)
ABLK = [op[1] for op in SCHED if op[0] == "A"]
RBLK = [op[1] for op in SCHED if op[0] == "D" and op[4] is not None]

_CACHE = {}


def _build(nloop=1, norm_f32r=True):
    nc = bacc.Bacc("TRN2", target_bir_lowering=False, debug=False)

    xt_d = nc.dram_tensor("xt", [D, B], BF16, kind="ExternalInput").ap()
    pt_d = nc.dram_tensor("pt", [D, CS], BF16, kind="ExternalInput").ap()
    xsb_d = nc.dram_tensor("xsb", [BS, D], F32, kind="ExternalInput").ap()
    pp_d = nc.dram_tensor("pp", [BS, D], F32, kind="ExternalInput").ap()
    s_d = nc.dram_tensor("s_out", [128, NACT], F32, kind="ExternalOutput").ap()
    d_d = nc.dram_tensor("d_out", [128, NRED], F32, kind="ExternalOutput").ap()
    pos_d = nc.dram_tensor("pos_out", [BS], F32, kind="ExternalOutput").ap()

    # Schraudolph scratch: 1024-wide, int32 write view + fp32 read view of
    # the same SBUF bytes (alloc_sbuf_tensor_at aliasing).
    arena = nc.alloc_sbuf_tensor("schr_arena", [128, 2 * WD], F32)
    addr = nc.lookup_mloc(arena).addr
    sints = nc.alloc_sbuf_tensor_at("schr_i", [128, 2 * WD], I32,
                                    offset=addr).ap()
    fview = nc.alloc_sbuf_tensor_at("schr_f", [128, 2 * WD], F32,
                                    offset=addr).ap()

    with tile.TileContext(nc) as tc:
        with (
            tc.tile_pool(name="res", bufs=1) as res,
            tc.tile_pool(name="sml", bufs=2) as sml,
            tc.tile_pool(name="pa", bufs=2, space="PSUM") as pa,
            tc.tile_pool(name="pd", bufs=2, space="PSUM") as pd,
        ):
            xsb = res.tile([BS, D], F32, tag="xsb")
            pp = res.tile([BS, D], F32, tag="pp")
            nc.sync.dma_start(xsb[:], xsb_d[:])
            nc.sync.dma_start(pp[:], pp_d[:])
            xtr2 = res.tile([2 * D, B], BF16, tag="xtr2")
            nc.sync.dma_start(xtr2[0:D, :], xt_d[:])
            nc.sync.dma_start(xtr2[D:2 * D, :], xt_d[:])
            ptn2 = res.tile([2 * D, CS], BF16, tag="ptn2")
            nc.sync.dma_start(ptn2[0:D, :], pt_d[:])
            nc.sync.dma_start(ptn2[D:2 * D, :], pt_d[:])

            def mms(ps, m, off, w):
                """Row-packed 512-col matmul pairs covering [off, off+w)."""
                c0 = 0
                h = 0
                while c0 < w:
                    cw = min(512, w - c0)
                    lo = D * h
                    nc.tensor.matmul(ps[:, c0:c0 + cw],
                                     xtr2[lo:lo + D, 128 * m:128 * (m + 1)],
                                     ptn2[lo:lo + D, off + c0:off + c0 + cw],
                                     start=True, stop=True,
                                     tile_position=(lo, 0))
                    h ^= 1
                    c0 += cw

            def body():
                # positive term pos = 2 x.p_hat_t
                xp = sml.tile([BS, D], F32, tag="xp")
                nc.vector.tensor_tensor(xp[:], xsb[:], pp[:], op=MULT)
                pos = sml.tile([BS, 1], F32, tag="pos")
                nc.vector.reduce_sum(pos[:], xp[:], axis=AX)
                nc.sync.dma_start(pos_d[:], pos[:, 0])

                sums = sml.tile([128, NACT], F32, tag="sums")
                dsums = sml.tile([128, NRED], F32, tag="dsums")
                for op in SCHED:
                    if op[0] == "A":
                        _, m, off, w, j = op
                        ps = pa.tile([128, WA], F32, tag="ps")
                        mms(ps, m, off, w)
                        # exp in place over its own PSUM input: the write of
                        # element c trails its read, and a PSUM-only operand
                        # set avoids the slower SBUF access-latency path.
                        nc.scalar.activation(ps[:, 0:w], ps[:, 0:w], EXP,
                                             accum_out=sums[:, j:j + 1])
                    else:
                        _, m, off, hoff, red = op
                        psd = pd.tile([128, WD], F32, tag="psd")
                        mms(psd, m, off, WD)
                        nc.vector.tensor_scalar(sints[:, hoff:hoff + WD],
                                                psd[:], SCH_A, SCH_B,
                                                op0=MULT, op1=ADD)
                        if red is not None:
                            r = nc.vector.reduce_sum(
                                dsums[:, red:red + 1],
                                fview[:, 0:2 * WD], axis=AX)
                            # fview aliases sints' bytes but is a different
                            # tensor; declare the int view as an extra input
                            # so RAW (ts->reduce) and WAR (reduce->next ts)
                            # ordering holds.
                            ri = r.ins
                            ri.ins = list(ri.ins) + [
                                nc.vector.lower_ap(sints[:, 0:2 * WD])]
                nc.sync.dma_start(s_d[:], sums[:])
                nc.sync.dma_start(d_d[:], dsums[:])

            if nloop == 1:
                body()
            else:
                with tc.For_i(0, nloop, 1):
                    body()

    nc.compile()
    return nc


def _get_nc(nloop=1, norm_f32r=True):
    key = (nloop, norm_f32r)
    if key not in _CACHE:
        _CACHE[key] = _build(nloop, norm_f32r)
    return _CACHE[key]


def _in_maps(xs, ts, proxies):
    xs = np.ascontiguousarray(np.asarray(xs), dtype=np.float32)
    proxies = np.ascontiguousarray(np.asarray(proxies), dtype=np.float32)
    ts = np.asarray(ts).astype(np.int64)
    norms = np.sqrt((proxies.astype(np.float64) ** 2).sum(1))
    p2 = proxies * (2.0 / np.maximum(norms, 1e-12))[:, None].astype(np.float32)
    pt_all = np.ascontiguousarray(p2.T.astype(ml_dtypes.bfloat16))
    xt = np.ascontiguousarray(xs.T.astype(ml_dtypes.bfloat16))
    ppos = p2[ts]
    maps = []
    for c in range(NCORES):
        maps.append({
            "xt": xt,
            "pt": np.ascontiguousarray(pt_all[:, c * CS:(c + 1) * CS]),
            "xsb": xs[c * BS:(c + 1) * BS],
            "pp": np.ascontiguousarray(ppos[c * BS:(c + 1) * BS]),
        })
    return maps


def _combine(results, ts=None):
    s = np.zeros(B, dtype=np.float64)
    pos = np.zeros(B, dtype=np.float64)
    for c in range(NCORES):
        so = results[c]["s_out"].astype(np.float64)   # [128, NACT]
        do = results[c]["d_out"].astype(np.float64)   # [128, NRED]
        acc = np.zeros((NBLK, 128), dtype=np.float64)
        for j, m in enumerate(ABLK):
            acc[m] += so[:, j]
        for j, m in enumerate(RBLK):
            acc[m] += do[:, j]
        s += acc.reshape(B)
        pos[c * BS:(c + 1) * BS] = results[c]["pos_out"].astype(np.float64)
    r = s - np.exp(pos)
    loss = np.mean(-pos + np.log(r))
    return np.asarray(loss, dtype=np.float32)


def kernel(xs, ts, proxies):
    nc = _get_nc()
    maps = _in_maps(xs, ts, proxies)
    results = run_bass_kernel_spmd(nc, maps, list(range(NCORES))).results
    return _combine(results, ts)


if __name__ == "__main__":
    rng = np.random.default_rng(0)
    xs = rng.standard_normal((B, D)).astype(np.float32)
    ts = rng.integers(0, C, B)
    proxies = rng.standard_normal((C, D)).astype(np.float32)
    print(kernel(xs=xs, ts=ts, proxies=proxies))
